# revision 1
# baseline (speedup 1.0000x reference)
"""Multi-head attention (B=4, L=2048, D=768, H=12) on 8 TRN2 NeuronCores.

Sharding: core c handles batch b=c//2, head-group g=c%2 (6 heads each).
Each core computes qkv projection for its heads, flash-style attention
(scores never leave SBUF/PSUM), and its partial output projection.
Host sums the two partial projections per batch element and adds b_out.

On-chip layout is transposed ([feature, seq]); the host supplies x
pre-transposed per batch and transposes the per-core output back.
All matmul operands are bf16 (fp32 accumulation in PSUM); softmax skips
max-subtraction (logits are provably tiny for this problem's scale).

Emission order interleaves the qk projection of head-pair p+1 between
attention head-pairs so projection matmuls fill PE idle slots during the
ACT(exp)-bound attention phase. One unified PSUM tag ("ss") is shared by
all projection/score matmuls: 2 slots x 2 banks, plus 4 banks for the
attention output accumulator.
"""

import sys

sys.path.insert(0, "/opt/trn_rl_repo")

import numpy as np

B, L, D = 4, 2048, 768
H, DH = 12, 64
HPC = 6  # heads per core
N_CORES = 8
QK = 2 * HPC * DH  # 768 qk-projection rows per core
V_W = HPC * (DH + 1)  # v tile width: 6 heads x (64 dims + ones col)

_state = None


def _emit(nc, tc, tile, mybir, bass, debug=False, nrep=1):
    f32 = mybir.dt.float32
    bf16 = mybir.dt.bfloat16
    Exp = mybir.ActivationFunctionType.Exp

    xT = nc.declare_dram_parameter("xT", [D, L], bf16, isOutput=False)
    w_qk = nc.declare_dram_parameter("w_qk", [D, QK], bf16, isOutput=False)
    b_qk = nc.declare_dram_parameter("b_qk", [128, QK // 128], f32, isOutput=False)
    w_v = nc.declare_dram_parameter("w_v", [D, HPC * DH], bf16, isOutput=False)
    b_v = nc.declare_dram_parameter("b_v", [1, HPC * DH], bf16, isOutput=False)
    w_out = nc.declare_dram_parameter("w_out", [HPC * DH, D], bf16, isOutput=False)
    outT = nc.declare_dram_parameter("outT", [D, L], f32, isOutput=True)
    if debug:
        qkt_d = nc.declare_dram_parameter("qkt_d", [64, 12 * L], bf16, isOutput=True)
        v_d = nc.declare_dram_parameter("v_d", [128, 16 * V_W], bf16, isOutput=True)
        rb_d = nc.declare_dram_parameter("rb_d", [128, HPC * L], f32, isOutput=True)
        at_d = nc.declare_dram_parameter("at_d", [128, 3 * L], bf16, isOutput=True)
        rs_d = nc.declare_dram_parameter("rs_d", [1, HPC * L], f32, isOutput=True)

    KT = D // 128  # 6 contraction tiles for the qkv projection
    NVC = HPC * DH  # 384 v columns
    LT = L // 128  # 16 seq tiles

    from contextlib import ExitStack, nullcontext

    with tc.For_i(0, nrep, 1) if nrep > 1 else nullcontext(), ExitStack() as ctx:
        persist = ctx.enter_context(tc.tile_pool(name="persist", bufs=1))
        qkt = persist.tile([64, 2 * HPC, L], bf16, tag="qkt")
        v = persist.tile([128, LT, V_W], bf16, tag="v")
        at = persist.tile([128, HPC * DH // 128, L], bf16, tag="at")
        rb = persist.tile([128, HPC, L], f32, tag="rb")
        wout_s = persist.tile([128, HPC * DH // 128, D], bf16, tag="wout")
        bqk_s = persist.tile([128, QK // 128], f32, tag="bqk")
        ones = persist.tile([1, 128], bf16, tag="ones")

        xt = persist.tile([128, KT, L], bf16, tag="xt")
        wqk_s = persist.tile([128, KT, QK], bf16, tag="wqk")
        wv_s = persist.tile([128, KT, NVC], bf16, tag="wv")
        bv_s = persist.tile([1, NVC], bf16, tag="bv")

        for k in range(KT):
            nc.sync.dma_start(out=xt[:, k, :], in_=xT[k * 128 : (k + 1) * 128, :])
            nc.scalar.dma_start(
                out=wv_s[:, k, :], in_=w_v[k * 128 : (k + 1) * 128, :]
            )
            nc.gpsimd.dma_start(
                out=wqk_s[:, k, :], in_=w_qk[k * 128 : (k + 1) * 128, :]
            )
        nc.sync.dma_start(out=bv_s, in_=b_v[:, :])
        nc.sync.dma_start(out=bqk_s, in_=b_qk[:, :])
        nc.sync.dma_start(out=wout_s, in_=w_out.rearrange("(t p) d -> p t d", p=128))
        nc.vector.memset(ones, 1.0)
        v_heads = v.rearrange("p t (h c) -> p t h c", h=HPC)
        nc.vector.memset(v_heads[:, :, :, DH : DH + 1], 1.0)

        sp = ctx.enter_context(tc.tile_pool(name="sp", bufs=3, space="PSUM"))
        op = ctx.enter_context(tc.tile_pool(name="op", bufs=1, space="PSUM"))
        ep = ctx.enter_context(tc.tile_pool(name="ep", bufs=4))
        rp = ctx.enter_context(tc.tile_pool(name="rp", bufs=1))
        ostage = ctx.enter_context(tc.tile_pool(name="ostage", bufs=2))

        def v_proj_tile(mt):
            if True:
                ss_t = sp.tile([128, 1024], f32, tag="ss")
                psv = ss_t[:, :NVC]
                for k in range(KT):
                    nc.tensor.matmul(
                        psv,
                        lhsT=xt[:, k, mt * 128 : (mt + 1) * 128],
                        rhs=wv_s[:, k, :],
                        start=(k == 0),
                        stop=False,
                    )
                nc.tensor.matmul(
                    psv, lhsT=ones[0:1, :], rhs=bv_s[0:1, :], start=False, stop=True
                )
                nc.vector.tensor_copy(
                    out=v_heads[:, mt, :, 0:DH],
                    in_=psv.rearrange("p (h c) -> p h c", c=DH),
                )

        def qk_proj_chunk(m, c):
            ss_t = sp.tile([128, 1024], f32, tag="ss")
            ps = ss_t[:, :512]
            for k in range(KT):
                nc.tensor.matmul(
                    ps,
                    lhsT=wqk_s[:, k, m * 128 : (m + 1) * 128],
                    rhs=xt[:, k, c * 512 : (c + 1) * 512],
                    start=(k == 0),
                    stop=(k == KT - 1),
                )
            for half in range(2):
                hh = 2 * m + half if m < 3 else HPC + 2 * (m - 3) + half
                nc.vector.tensor_scalar_add(
                    out=qkt[:, hh, c * 512 : (c + 1) * 512],
                    in0=ps[64 * half : 64 * half + 64, :],
                    scalar1=bqk_s[64 * half : 64 * half + 64, m : m + 1],
                )

        def qk_proj_pair(hp):
            for m in (hp, 3 + hp):
                for c in range(4):
                    qk_proj_chunk(m, c)

        def attn_head(h, fillers=(), stride=4):
            fillers = list(fillers)
            off = 64 * (h % 2)
            qt = qkt[:, h, :]
            kt = qkt[:, HPC + h, :]
            HL = L // 2
            for lqh in range(2):
                po = op.tile([65, HL], f32, tag="po")
                av_prev = None
                for mk in range(LT):
                    if fillers and (lqh * LT + mk) % stride == 0:
                        fillers.pop(0)()
                    ss = sp.tile([128, 1024], f32, tag="ss")
                    for j in range(2):
                        nc.tensor.matmul(
                            ss[:, j * 512 : (j + 1) * 512],
                            lhsT=kt[:, mk * 128 : (mk + 1) * 128],
                            rhs=qt[:, lqh * HL + j * 512 : lqh * HL + (j + 1) * 512],
                            start=True,
                            stop=True,
                        )
                    ex = ep.tile([128, 1024], bf16, tag="ex")
                    nc.scalar.activation(out=ex, in_=ss, func=Exp, scale=0.125)
                    if av_prev is not None:
                        av_prev()

                    def av_now(mk=mk, ex=ex):
                        for j in range(2):
                            nc.tensor.matmul(
                                po[:, j * 512 : (j + 1) * 512],
                                lhsT=v[:, mk, h * 65 : (h + 1) * 65],
                                rhs=ex[:, j * 512 : (j + 1) * 512],
                                start=(mk == 0),
                                stop=(mk == LT - 1),
                            )

                    av_prev = av_now
                av_prev()
                sl = slice(lqh * HL, (lqh + 1) * HL)
                rsh = rp.tile([1, HL], f32, tag="rsh")
                nc.vector.tensor_copy(out=rsh, in_=po[64:65, :])
                nc.vector.tensor_copy(
                    out=at[off : off + 64, h // 2, sl], in_=po[0:64, :]
                )
                nc.vector.reciprocal(out=rsh, in_=rsh)
                if debug:
                    nc.sync.dma_start(
                        out=rs_d[0:1, h * L + lqh * HL : h * L + (lqh + 1) * HL],
                        in_=rsh,
                    )
                nc.gpsimd.partition_broadcast(rb[:, h, sl], rsh[0:1, :], channels=128)
                nc.vector.tensor_mul(
                    out=at[off : off + 64, h // 2, sl],
                    in0=at[off : off + 64, h // 2, sl],
                    in1=rb[off : off + 64, h, sl],
                )

        def qkf(m, c):
            return lambda: qk_proj_chunk(m, c)

        def vf(mt):
            return lambda: v_proj_tile(mt)

        # prelude: just enough for head 0 to start
        for mt in range(4):
            v_proj_tile(mt)
        for c in range(4):
            qk_proj_chunk(0, c)
        qk_proj_chunk(3, 0)
        # the rest rides along as fillers inside the attention chunk loops
        f0 = [vf(4), qkf(3, 1), vf(5), vf(6), vf(7), qkf(3, 2), vf(8), vf(9),
              vf(10), vf(11), qkf(3, 3), vf(12), vf(13), vf(14), vf(15)]
        f1 = [qkf(m, c) for m in (1, 4) for c in range(4)]
        f2 = [qkf(m, c) for m in (2, 5) for c in range(4)]
        attn_head(0, f0, stride=1)
        attn_head(1, f1, stride=4)
        attn_head(2, f2, stride=4)
        attn_head(3)
        attn_head(4)
        attn_head(5)

        if debug:
            nc.sync.dma_start(out=qkt_d.rearrange("p (m l) -> p m l", m=12), in_=qkt)
            nc.sync.dma_start(out=v_d.rearrange("p (t w) -> p t w", t=16), in_=v)
            nc.sync.dma_start(out=rb_d.rearrange("p (m l) -> p m l", m=HPC), in_=rb)
            nc.sync.dma_start(out=at_d.rearrange("p (m l) -> p m l", m=3), in_=at)

        # output projection: psum -> sbuf staging -> dram
        for m in range(D // 128):
            for c in range(4):
                ss_t = sp.tile([128, 1024], f32, tag="ss")
                pso = ss_t[:, :512]
                for k in range(HPC * DH // 128):
                    nc.tensor.matmul(
                        pso,
                        lhsT=wout_s[:, k, m * 128 : (m + 1) * 128],
                        rhs=at[:, k, c * 512 : (c + 1) * 512],
                        start=(k == 0),
                        stop=(k == HPC * DH // 128 - 1),
                    )
                ot = ostage.tile([128, 512], f32, tag="ot")
                nc.vector.tensor_copy(out=ot, in_=pso)
                nc.sync.dma_start(
                    out=outT[m * 128 : (m + 1) * 128, c * 512 : (c + 1) * 512],
                    in_=ot,
                )


def _build(debug=False, nrep=1):
    global _state
    if not debug and nrep == 1 and _state is not None:
        return _state
    import concourse.bacc as bacc
    import concourse.tile as tile
    import concourse.bass as bass
    from concourse import mybir

    nc = bacc.Bacc("TRN2", target_bir_lowering=False)
    with tile.TileContext(nc) as tc:
        _emit(nc, tc, tile, mybir, bass, debug=debug, nrep=nrep)
    nc.compile()
    if debug or nrep != 1:
        return nc
    _state = nc
    return nc


def make_in_maps(x, W_qkv, b_qkv, W_out):
    """Host-side sharding: per-core input dict."""
    import ml_dtypes

    bf = ml_dtypes.bfloat16
    x = np.asarray(x, np.float32).astype(bf)
    W_qkv = np.asarray(W_qkv, np.float32).astype(bf)
    b_qkv = np.asarray(b_qkv, np.float32)
    W_out = np.asarray(W_out, np.float32).astype(bf)
    in_maps = []
    for c in range(N_CORES):
        b, g = divmod(c, 2)
        qs = slice(384 * g, 384 * g + 384)
        ks = slice(768 + 384 * g, 768 + 384 * g + 384)
        vs = slice(1536 + 384 * g, 1536 + 384 * g + 384)
        bqk = np.concatenate([b_qkv[qs], b_qkv[ks]])
        in_maps.append(
            {
                "xT": np.ascontiguousarray(x[b].T),
                "w_qk": np.ascontiguousarray(
                    np.concatenate([W_qkv[:, qs], W_qkv[:, ks]], axis=1)
                ),
                "b_qk": np.ascontiguousarray(bqk.reshape(QK // 128, 128).T),
                "w_v": np.ascontiguousarray(W_qkv[:, vs]),
                "b_v": np.ascontiguousarray(b_qkv[vs][None, :].astype(bf)),
                "w_out": np.ascontiguousarray(W_out[384 * g : 384 * g + 384, :]),
            }
        )
    return in_maps


def gather(results, b_out):
    """Host-side unshard: sum the two partial projections per batch + bias."""
    b_out = np.asarray(b_out, np.float32)
    out = np.empty((B, L, D), np.float32)
    for b in range(B):
        yt = results[2 * b]["outT"] + results[2 * b + 1]["outT"]
        out[b] = yt.T + b_out
    return out


def kernel(x, W_qkv, b_qkv, W_out, b_out):
    from concourse.bass_utils import run_bass_kernel_spmd

    nc = _build()
    in_maps = make_in_maps(x, W_qkv, b_qkv, W_out)
    res = run_bass_kernel_spmd(nc, in_maps, list(range(N_CORES)))
    return gather(res.results, b_out)



# revision 2
# speedup vs baseline: 1.1135x; 1.1135x over previous
"""Multi-head attention (B=4, L=2048, D=768, H=12) on 8 TRN2 NeuronCores — v3.

Sharding: core c handles batch b=c//2, head-group g=c%2 (6 heads each).

Design (HW-calibrated against microbenchmarks):
- 64-contraction matmuls stream at half rate on TRN2 (measured 525ns vs
  327ns per [*,512] matmul). Scores therefore use a checkerboard layout:
  qkt2[0:64]=q, qkt2[64:128]=dup(q); kpad2 holds K block-diagonally
  (even 64-key blocks in rows 0:64, odd blocks in rows 64:128, zeros
  elsewhere), so each score matmul is a full-rate plain bf16
  [128,128]x[128,512] producing 128 consecutive keys' scores.
- AV runs as fp8e4m3 DoubleRow over mk-tile pairs (ex and V in fp8):
  one matmul consumes 256 keys, 2x the bf16 rate. The ones-column in V
  accumulates the softmax denominator (fp32 in PSUM).
- W_qkv columns interleaved per head ([q_h|k_h] per 128-col chunk) so the
  projection PSUM->SBUF move is one tensor_scalar_add per chunk.
- epilogue fused: reciprocal directly from the PSUM ones-row, gpsimd
  partition_broadcast, single tensor_mul PSUM x bcast -> `at` (bf16).
- loop order qc-outer so out-projection chunks run as fillers early.
"""

import sys

sys.path.insert(0, "/opt/trn_rl_repo")

import numpy as np

B, L, D = 4, 2048, 768
H, DH = 12, 64
HPC = 6  # heads per core
N_CORES = 8
QK = 768  # q+k features per core (6 heads x (64+64))
NVC = HPC * DH  # 384 v columns
VH = 68  # per-head v slot: 64 dims + ones col + zero col + 2 pad (4B-aligned)
KT = D // 128  # 6 contraction tiles
LT = L // 128  # 16 key tiles
NQC = 4  # query chunks of 512

_state = None


def _emit(nc, tc, tile, mybir, bass, nrep=1):
    f32 = mybir.dt.float32
    bf16 = mybir.dt.bfloat16
    fp8 = mybir.dt.float8e4
    Exp = mybir.ActivationFunctionType.Exp
    DR = mybir.MatmulPerfMode.DoubleRow

    xT = nc.declare_dram_parameter("xT", [D, L], bf16, isOutput=False)
    w_qk = nc.declare_dram_parameter("w_qk", [D, QK], bf16, isOutput=False)
    b_qk = nc.declare_dram_parameter("b_qk", [128, HPC], f32, isOutput=False)
    w_v = nc.declare_dram_parameter("w_v", [D, NVC], bf16, isOutput=False)
    b_v = nc.declare_dram_parameter("b_v", [1, NVC], bf16, isOutput=False)
    w_out = nc.declare_dram_parameter("w_out", [NVC, D], bf16, isOutput=False)
    outT = nc.declare_dram_parameter("outT", [D, L], f32, isOutput=True)

    from contextlib import ExitStack, nullcontext

    with tc.For_i(0, nrep, 1) if nrep > 1 else nullcontext(), ExitStack() as ctx:
        persist = ctx.enter_context(tc.tile_pool(name="persist", bufs=1))
        xt = persist.tile([128, KT, L], bf16, tag="xt")
        wqk_s = persist.tile([128, KT, QK], bf16, tag="wqk")
        wv_s = persist.tile([128, KT, NVC], bf16, tag="wv")
        bqk_s = persist.tile([128, HPC], f32, tag="bqk")
        bv_s = persist.tile([1, NVC], bf16, tag="bv")
        wout_s = persist.tile([128, NVC // 128, D], bf16, tag="wout")
        # qkt2: rows 0:64 = q_h, rows 64:128 = k_h after projection, then
        # overwritten with dup(q_h) once k is relayouted into kpad2.
        qkt2 = persist.tile([128, HPC, L], bf16, tag="qkt2")
        # kpad2 checkerboard: col j holds K[:, j] in rows 0:64 if (j//64)
        # even else rows 64:128; the complementary cells are zero.
        kpad2 = persist.tile([128, HPC, L], bf16, tag="kpad2")
        # v8: per mk-tile 512-byte stride (dual-fp8 LDWEIGHTS requires the
        # DoubleRow group stride seen in validated layouts); head h occupies
        # cols [68h, 68h+66): 64 dims + ones + zero.
        v8 = persist.tile([128, LT, 512], fp8, tag="v8")
        at = persist.tile([128, NVC // 128, L], bf16, tag="at")
        ones = persist.tile([1, 128], bf16, tag="ones")

        for k in range(KT):
            nc.sync.dma_start(out=xt[:, k, :], in_=xT[k * 128 : (k + 1) * 128, :])
            nc.scalar.dma_start(out=wv_s[:, k, :], in_=w_v[k * 128 : (k + 1) * 128, :])
            nc.gpsimd.dma_start(
                out=wqk_s[:, k, :], in_=w_qk[k * 128 : (k + 1) * 128, :]
            )
        nc.sync.dma_start(out=bv_s, in_=b_v[:, :])
        nc.sync.dma_start(out=bqk_s, in_=b_qk[:, :])
        nc.sync.dma_start(out=wout_s, in_=w_out.rearrange("(t p) d -> p t d", p=128))
        nc.vector.memset(ones, 1.0)
        v8h = v8[:, :, 0 : HPC * VH].rearrange("p t (h c) -> p t h c", c=VH)
        nc.vector.memset(v8h[:, :, :, DH : DH + 1], 1.0)
        nc.vector.memset(v8h[:, :, :, DH + 1 : VH], 0.0)

        sp = ctx.enter_context(tc.tile_pool(name="sp", bufs=3, space="PSUM"))
        op = ctx.enter_context(tc.tile_pool(name="op", bufs=2, space="PSUM"))
        ep = ctx.enter_context(tc.tile_pool(name="ep", bufs=4))
        rp = ctx.enter_context(tc.tile_pool(name="rp", bufs=2))
        bp = ctx.enter_context(tc.tile_pool(name="bp", bufs=2))
        ostage = ctx.enter_context(tc.tile_pool(name="ostage", bufs=2))

        def v_proj_tile(mt):
            ss_t = sp.tile([128, 1024], f32, tag="ss")
            psv = ss_t[:, 0:NVC]
            for k in range(KT):
                nc.tensor.matmul(
                    psv,
                    lhsT=xt[:, k, mt * 128 : (mt + 1) * 128],
                    rhs=wv_s[:, k, :],
                    start=(k == 0),
                    stop=False,
                )
            nc.tensor.matmul(
                psv, lhsT=ones[0:1, :], rhs=bv_s[0:1, :], start=False, stop=True
            )
            nc.vector.tensor_copy(
                out=v8h[:, mt, :, 0:DH],
                in_=psv.rearrange("p (h c) -> p h c", c=DH),
            )

        def qk_proj_chunk(j, c):
            ss_t = sp.tile([128, 1024], f32, tag="ss")
            ps = ss_t[:, 0:512]
            for k in range(KT):
                nc.tensor.matmul(
                    ps,
                    lhsT=wqk_s[:, k, j * 128 : (j + 1) * 128],
                    rhs=xt[:, k, c * 512 : (c + 1) * 512],
                    start=(k == 0),
                    stop=(k == KT - 1),
                )
            nc.vector.tensor_scalar_add(
                out=qkt2[:, j, c * 512 : (c + 1) * 512],
                in0=ps,
                scalar1=bqk_s[:, j : j + 1],
            )

        def qk_relayout(j):
            """kpad2 checkerboard from qkt2 rows 64:128; q-dup into 64:128."""
            kv = kpad2[:, j, :].rearrange("p (t n) -> p t n", n=64)
            qv = qkt2[:, j, :].rearrange("p (t n) -> p t n", n=64)
            # even 64-blocks of K -> rows 0:64; odd blocks -> rows 64:128
            nc.vector.tensor_copy(out=kv[0:64, 0::2, :], in_=qv[64:128, 0::2, :])
            nc.vector.tensor_copy(out=kv[64:128, 1::2, :], in_=qv[64:128, 1::2, :])
            # zero the complementary checkerboard cells
            nc.vector.memset(kv[64:128, 0::2, :], 0.0)
            nc.vector.memset(kv[0:64, 1::2, :], 0.0)
            # duplicate q into rows 64:128 (after k was copied out)
            nc.vector.tensor_copy(
                out=qkt2[64:128, j, :], in_=qkt2[0:64, j, :]
            )

        def attn_block(h, qc, fillers=(), stride=1):
            """Attention for head h, query chunk qc (512 queries)."""
            fillers = list(fillers)
            qs = slice(qc * 512, (qc + 1) * 512)
            po = op.tile([66, 512], f32, tag="po")
            av_prev = None
            for p in range(LT // 2):
                if fillers and p % stride == 0:
                    fillers.pop(0)()
                ss_t = sp.tile([128, 1024], f32, tag="ss")
                ssv = ss_t.rearrange("p (g n) -> p g n", g=2)
                for g in range(2):
                    mk = 2 * p + g
                    nc.tensor.matmul(
                        ssv[:, g, :],
                        lhsT=kpad2[:, h, mk * 128 : (mk + 1) * 128],
                        rhs=qkt2[:, h, qs],
                        start=True,
                        stop=True,
                    )
                ex = ep.tile([128, 2, 512], fp8, tag="ex")
                nc.scalar.activation(out=ex, in_=ssv, func=Exp, scale=0.125)
                if av_prev is not None:
                    av_prev()

                def av_now(p=p, ex=ex):
                    nc.tensor.matmul(
                        po,
                        lhsT=v8[:, 2 * p : 2 * p + 2, h * VH : h * VH + 66],
                        rhs=ex,
                        start=(p == 0),
                        stop=(p == LT // 2 - 1),
                        perf_mode=DR,
                    )

                av_prev = av_now
            while fillers:
                fillers.pop(0)()
            av_prev()
            rsh = rp.tile([1, 512], f32, tag="rsh")
            nc.vector.reciprocal(out=rsh, in_=po[64:65, :])
            rb = bp.tile([128, 512], f32, tag="rb")
            nc.gpsimd.partition_broadcast(rb, rsh[0:1, :], channels=128)
            off = 64 * (h % 2)
            nc.vector.tensor_mul(
                out=at[off : off + 64, h // 2, qs],
                in0=po[0:64, :],
                in1=rb[off : off + 64, :],
            )

        def out_proj(m, c):
            ss_t = sp.tile([128, 1024], f32, tag="ss")
            pso = ss_t[:, 0:512]
            for k in range(NVC // 128):
                nc.tensor.matmul(
                    pso,
                    lhsT=wout_s[:, k, m * 128 : (m + 1) * 128],
                    rhs=at[:, k, c * 512 : (c + 1) * 512],
                    start=(k == 0),
                    stop=(k == NVC // 128 - 1),
                )
            ot = ostage.tile([128, 512], f32, tag="ot")
            nc.vector.tensor_copy(out=ot, in_=pso)
            nc.sync.dma_start(
                out=outT[m * 128 : (m + 1) * 128, c * 512 : (c + 1) * 512],
                in_=ot,
            )

        def qkf(j, c):
            return lambda: qk_proj_chunk(j, c)

        def qkr(j):
            return lambda: qk_relayout(j)

        def vf(mt):
            return lambda: v_proj_tile(mt)

        def opf(m, c):
            return lambda: out_proj(m, c)

        def qk_head_units(j):
            return [qkf(j, c) for c in range(NQC)] + [qkr(j)]

        # prelude: v tiles 0-7, heads 0 and 1 fully projected + relayouted
        for mt in range(8):
            v_proj_tile(mt)
        for j in (0, 1):
            for c in range(NQC):
                qk_proj_chunk(j, c)
            qk_relayout(j)

        for qc in range(NQC):
            for h in range(HPC):
                if qc == 0:
                    if h == 0:
                        fillers = [vf(mt) for mt in range(8, 16)]
                    elif h <= 4:
                        fillers = qk_head_units(h + 1)
                    else:
                        fillers = []
                else:
                    fillers = [opf(h, qc - 1)]
                attn_block(h, qc, fillers, stride=1)
        for m in range(D // 128):
            out_proj(m, NQC - 1)


def _build(nrep=1):
    global _state
    if nrep == 1 and _state is not None:
        return _state
    import concourse.bacc as bacc
    import concourse.tile as tile
    import concourse.bass as bass
    from concourse import mybir

    nc = bacc.Bacc("TRN2", target_bir_lowering=False)
    with tile.TileContext(nc) as tc:
        _emit(nc, tc, tile, mybir, bass, nrep=nrep)
    nc.compile()
    if nrep != 1:
        return nc
    _state = nc
    return nc


def make_in_maps(x, W_qkv, b_qkv, W_out):
    """Host-side sharding: per-core input dict (interleaved q|k layout)."""
    import ml_dtypes

    bf = ml_dtypes.bfloat16
    x = np.asarray(x, np.float32).astype(bf)
    W_qkv = np.asarray(W_qkv, np.float32)
    b_qkv = np.asarray(b_qkv, np.float32)
    W_out = np.asarray(W_out, np.float32).astype(bf)
    in_maps = []
    for c in range(N_CORES):
        b, g = divmod(c, 2)
        wqk_cols = []
        bqk_cols = []
        for j in range(HPC):
            hg = g * HPC + j
            qs = slice(hg * DH, (hg + 1) * DH)
            ks = slice(D + hg * DH, D + (hg + 1) * DH)
            wqk_cols.append(W_qkv[:, qs])
            wqk_cols.append(W_qkv[:, ks])
            bqk_cols.append(np.concatenate([b_qkv[qs], b_qkv[ks]]))
        vs = slice(2 * D + g * NVC, 2 * D + (g + 1) * NVC)
        in_maps.append(
            {
                "xT": np.ascontiguousarray(x[b].T),
                "w_qk": np.ascontiguousarray(
                    np.concatenate(wqk_cols, axis=1).astype(bf)
                ),
                "b_qk": np.ascontiguousarray(np.stack(bqk_cols, axis=1)),
                "w_v": np.ascontiguousarray(W_qkv[:, vs].astype(bf)),
                "b_v": np.ascontiguousarray(b_qkv[vs][None, :].astype(bf)),
                "w_out": np.ascontiguousarray(W_out[g * NVC : (g + 1) * NVC, :]),
            }
        )
    return in_maps


def gather(results, b_out):
    """Host-side unshard: sum the two partial projections per batch + bias."""
    b_out = np.asarray(b_out, np.float32)
    out = np.empty((B, L, D), np.float32)
    for b in range(B):
        yt = results[2 * b]["outT"] + results[2 * b + 1]["outT"]
        out[b] = yt.T + b_out
    return out


def kernel(x, W_qkv, b_qkv, W_out, b_out):
    from concourse.bass_utils import run_bass_kernel_spmd

    nc = _build()
    in_maps = make_in_maps(x, W_qkv, b_qkv, W_out)
    res = run_bass_kernel_spmd(nc, in_maps, list(range(N_CORES)))
    return gather(res.results, b_out)
